# revision 2
# baseline (speedup 1.0000x reference)
"""HMQ-quantized MLP (fc1 -> exact GELU -> fc2) on 8 TRN2 NeuronCores.

Strategy: data-parallel over the 16384 token rows (2048 rows/core).
The int8 fake-quant values are integers in [-127, 127], exactly representable
in bf16, and all dot-product partial sums stay far below 2^24 -- so the
dequantized GEMMs are computed EXACTLY as bf16 integer matmuls on the PE
array with fp32 PSUM accumulation, then scaled by s_a*s_w.  Rounding uses
the +/-1.5*2^23 magic-constant trick (matches jnp.round round-half-even).

v2 changes vs baseline:
  - no on-device transposes at all (b1 pre-blocked on host) -> no fp32
    matmuls, which keeps the PE weight-load path on the fast mode.
  - x shard is staged in SBUF as bf16 during the abs-max pass and
    quantized in place after the scale collective (no second HBM read).
  - the hidden activation g is staged to DRAM in bf16 (half the traffic);
    its abs-max is taken from the f32 gelu output, so the global scale is
    exact; bf16 pre-rounding only perturbs a ~0.3% fraction of the
    round-to-int decisions (well within tolerance).
  - startup/transition pipelining: w1 chunks prefetch during the scale
    collective; x quantize is split across Scalar and Vector engines in
    matmul consumption order so fc1 starts ~2us after scales arrive.
"""

import numpy as np

import concourse.bass as bass
import concourse.mybir as mybir
import concourse.tile as tile
from concourse import bacc, bass_isa
from concourse.bass_utils import run_bass_kernel_spmd

F32 = mybir.dt.float32
BF16 = mybir.dt.bfloat16
ts = bass.ts

C_MAGIC = 1.5 * 2**23  # round-to-nearest-even for |v| < 2^22
QMAX = 127.0

NCORES = 8
B, T, D, H = 4, 4096, 1024, 4096
M = B * T            # 16384 total rows
S = M // NCORES      # 2048 rows per core

N_IC = D // 128      # 8  contraction chunks for fc1
N_OC = H // 128      # 32 output chunks for fc1 (hidden)
N_ST = S // 512      # 4  row tiles of 512
N_SC = S // 128      # 16 row chunks of 128
N_NC = H // 128      # 32 contraction chunks for fc2
N_JT = D // 512      # 2  output col tiles for fc2

Copy = mybir.ActivationFunctionType.Copy
Gelu = mybir.ActivationFunctionType.Gelu
X_AX = mybir.AxisListType.X
MAX = mybir.AluOpType.max
MULT = mybir.AluOpType.mult
SUB = mybir.AluOpType.subtract
ADD = mybir.AluOpType.add


def build():
    nc = bacc.Bacc("TRN2", target_bir_lowering=False, debug=False,
                   num_devices=NCORES)

    xts = nc.dram_tensor("xts", [D, S], F32, kind="ExternalInput")
    w1r = nc.dram_tensor("w1r", [128, N_IC, H], F32, kind="ExternalInput")
    w1s = nc.dram_tensor("w1s", [H // NCORES, D], F32, kind="ExternalInput")
    w2t = nc.dram_tensor("w2t", [H, D], F32, kind="ExternalInput")
    w2s = nc.dram_tensor("w2s", [D // NCORES, H], F32, kind="ExternalInput")
    b1a = nc.dram_tensor("b1a", [128, N_OC], F32, kind="ExternalInput")
    b2m = nc.dram_tensor("b2m", [1, D], F32, kind="ExternalInput")
    out = nc.dram_tensor("out", [S, D], F32, kind="ExternalOutput")

    with tile.TileContext(nc) as tc:
        with (
            tc.tile_pool(name="misc", bufs=1) as misc,
            tc.tile_pool(name="fstage", bufs=3) as fsp,
            tc.tile_pool(name="xst", bufs=16) as xsp,
            tc.tile_pool(name="qtmp", bufs=2) as qtp,
            tc.tile_pool(name="w1c", bufs=2) as w1sp,
            tc.tile_pool(name="w1q", bufs=2) as w1qp,
            tc.tile_pool(name="w2c", bufs=2) as w2sp,
            tc.tile_pool(name="w2q", bufs=1) as w2qp,
            tc.tile_pool(name="gout", bufs=3) as goutp,
            tc.tile_pool(name="gobf", bufs=3) as gobp,
            tc.tile_pool(name="gst", bufs=4) as gstp,
            tc.tile_pool(name="outp", bufs=2) as outp,
            tc.tile_pool(name="psum", bufs=8, space="PSUM") as psump,
            tc.tile_pool(name="dram", bufs=1, space="DRAM") as dramp,
        ):
            # ---------------- persistent DRAM intermediates ----------------
            gT = dramp.tile([H, S], BF16, tag="gT")
            cc1_in = dramp.tile([1, 4], F32, tag="cc1i")
            cc1_out = dramp.tile([NCORES, 4], F32, tag="cc1o")
            cc2_in = dramp.tile([1, 4], F32, tag="cc2i")
            cc2_out = dramp.tile([NCORES, 4], F32, tag="cc2o")

            # ---------------- bias prep (no transposes needed) -------------
            b1sb = misc.tile([128, N_OC], F32, tag="b1sb")
            nc.sync.dma_start(out=b1sb, in_=b1a[:, :])
            b2row = misc.tile([1, D], F32, tag="b2row")
            nc.sync.dma_start(out=b2row, in_=b2m[:, :])
            b2r = misc.tile([128, D], F32, tag="b2r")
            nc.gpsimd.partition_broadcast(b2r, b2row)

            # ------------- local abs-max pass; x staged to bf16 ------------
            # part1 cols: 0..15 x chunks | 16..19 w1 chunks | 20..23 w2 chunks
            part1 = misc.tile([128, 24], F32, tag="part1")
            xst = []  # 16 resident bf16 tiles [128,1024]; chunk k=(ic,half)
            for k in range(16):
                xc = fsp.tile([128, 1024], F32, tag="fs", name=f"xmax{k}")
                ic, hf = k // 2, k % 2
                nc.sync.dma_start(
                    out=xc,
                    in_=xts[ic * 128:(ic + 1) * 128, hf * 1024:(hf + 1) * 1024])
                nc.vector.tensor_reduce(out=part1[:, k:k + 1], in_=xc,
                                        axis=X_AX,
                                        op=MAX, apply_absolute_value=True)
                xb = xsp.tile([128, 1024], BF16, tag="xst", name=f"xst{k}")
                nc.vector.tensor_copy(xb, xc)
                xst.append(xb)
            # w1 shard [512, 1024] -> 4 chunks [128, 1024]
            for c in range(4):
                wc = fsp.tile([128, 1024], F32, tag="fs", name=f"w1m{c}")
                nc.sync.dma_start(out=wc, in_=w1s[c * 128:(c + 1) * 128, :])
                nc.vector.tensor_reduce(out=part1[:, 16 + c:17 + c], in_=wc,
                                        axis=X_AX,
                                        op=MAX, apply_absolute_value=True)
            # w2 shard [128, 4096] -> 4 chunks [128, 1024]
            for c in range(4):
                wc = fsp.tile([128, 1024], F32, tag="fs", name=f"w2m{c}")
                nc.sync.dma_start(out=wc, in_=w2s[:, c * 1024:(c + 1) * 1024])
                nc.vector.tensor_reduce(out=part1[:, 20 + c:21 + c], in_=wc,
                                        axis=X_AX,
                                        op=MAX, apply_absolute_value=True)

            # combine partials -> [x, w1, w2, w2] cols of arow
            arow = misc.tile([128, 4], F32, tag="arow")
            nc.vector.tensor_reduce(out=arow[:, 0:1], in_=part1[:, 0:16],
                                    axis=X_AX, op=MAX)
            nc.vector.tensor_reduce(out=arow[:, 1:2], in_=part1[:, 16:20],
                                    axis=X_AX, op=MAX)
            nc.vector.tensor_reduce(out=arow[:, 2:3], in_=part1[:, 20:24],
                                    axis=X_AX, op=MAX)
            nc.vector.tensor_copy(arow[:, 3:4], arow[:, 2:3])
            armax = misc.tile([128, 4], F32, tag="armax")
            nc.gpsimd.partition_all_reduce(armax, arow, channels=128,
                                           reduce_op=bass_isa.ReduceOp.max)

            # ------------- AllGather #1 -> global Mx, Mw1, Mw2 -------------
            nc.gpsimd.dma_start(out=cc1_in, in_=armax[0:1, :])
            nc.gpsimd.collective_compute(
                "AllGather", mybir.AluOpType.bypass,
                replica_groups=[list(range(NCORES))],
                ins=[cc1_in.opt()], outs=[cc1_out.opt()])
            g1g = misc.tile([NCORES, 4], F32, tag="g1g")
            nc.gpsimd.dma_start(out=g1g, in_=cc1_out[:, :])
            g1m = misc.tile([NCORES, 4], F32, tag="g1m")
            nc.gpsimd.partition_all_reduce(g1m, g1g, channels=NCORES,
                                           reduce_op=bass_isa.ReduceOp.max)
            g1 = misc.tile([128, 4], F32, tag="g1")
            nc.gpsimd.partition_broadcast(g1, g1m)

            # scl cols: 0 sx | 1 inv_sx | 2 sw1 | 3 inv_sw1 | 4 sw2 |
            #           5 inv_sw2 | 6 d1
            scl = misc.tile([128, 8], F32, tag="scl")
            for i in range(3):
                nc.vector.tensor_scalar(out=scl[:, 2 * i:2 * i + 1],
                                        in0=g1[:, i:i + 1],
                                        scalar1=1e-8, scalar2=1.0 / QMAX,
                                        op0=MAX, op1=MULT)
                nc.vector.reciprocal(scl[:, 2 * i + 1:2 * i + 2],
                                     scl[:, 2 * i:2 * i + 1])
            nc.vector.tensor_mul(scl[:, 6:7], scl[:, 0:1], scl[:, 2:3])

            # -------- quantize x in place (bf16), in MM consumption order --
            # even ic -> Scalar path, odd ic -> Vector path (both engines in
            # parallel so fc1's ic stream is fed at matmul pace).
            for ic in range(N_IC):
                for hf in range(2):
                    k = ic * 2 + hf
                    if ic % 2 == 0:
                        qt = qtp.tile([128, 2048], F32, tag="qt",
                                      name=f"xq{k}")
                        nc.scalar.activation(qt[:, 0:1024], xst[k], Copy,
                                             bias=C_MAGIC, scale=scl[:, 1:2])
                        nc.vector.tensor_scalar(out=xst[k], in0=qt[:, 0:1024],
                                                scalar1=C_MAGIC, scalar2=None,
                                                op0=SUB)
                    else:
                        qt = qtp.tile([128, 2048], F32, tag="qt",
                                      name=f"xq{k}")
                        nc.vector.tensor_scalar(out=qt[:, 0:1024], in0=xst[k],
                                                scalar1=scl[:, 1:2],
                                                scalar2=C_MAGIC,
                                                op0=MULT, op1=ADD)
                        nc.vector.tensor_scalar(out=xst[k], in0=qt[:, 0:1024],
                                                scalar1=C_MAGIC, scalar2=None,
                                                op0=SUB)

            # ---------------- fc1: h^T = w1q @ xq^T, gelu, stage g^T -------
            # w1 loads in 2-oc batches (1KB descriptor lines); w2 load +
            # quantize interleaved 2 chunks per batch to pace the 16 MB of
            # w2 reads across fc1.
            w2qT = w2qp.tile([128, N_NC, D], BF16, tag="w2q")
            gpart = misc.tile([128, N_OC * N_ST], F32, tag="gpart")
            for ocb in range(N_OC // 2):
                w1c = w1sp.tile([128, N_IC, 256], F32, tag="w1c",
                                name=f"w1c{ocb}")
                nc.sync.dma_start(out=w1c,
                                  in_=w1r[:, :, ocb * 256:(ocb + 1) * 256])
                w1q = w1qp.tile([128, N_IC, 256], BF16, tag="w1q",
                                name=f"w1q{ocb}")
                w1cf = w1c.rearrange("p a b -> p (a b)")
                nc.scalar.activation(w1cf, w1cf, Copy, bias=C_MAGIC,
                                     scale=scl[:, 3:4])
                nc.vector.tensor_scalar(
                    out=w1q.rearrange("p a b -> p (a b)"), in0=w1cf,
                    scalar1=C_MAGIC, scalar2=None, op0=SUB)
                for j in range(2):
                    oc = 2 * ocb + j
                    pts = [psump.tile([128, 512], F32, tag="mm",
                                      name=f"pt{oc}_{st}")
                           for st in range(N_ST)]
                    for ic in range(N_IC):
                        for st in range(N_ST):
                            nc.tensor.matmul(
                                pts[st],
                                lhsT=w1q[:, ic, j * 128:(j + 1) * 128],
                                rhs=xst[ic * 2 + st // 2][:,
                                                          (st % 2) * 512:
                                                          (st % 2) * 512 + 512],
                                start=(ic == 0), stop=(ic == N_IC - 1))
                    for st in range(N_ST):
                        go = goutp.tile([128, 512], F32, tag="gout",
                                        name=f"go{oc}_{st}")
                        nc.scalar.activation(go, pts[st], Gelu,
                                             bias=b1sb[:, oc:oc + 1],
                                             scale=scl[:, 6:7])
                        nc.vector.tensor_reduce(
                            out=gpart[:, oc * N_ST + st:oc * N_ST + st + 1],
                            in_=go, axis=X_AX, op=MAX,
                            apply_absolute_value=True)
                        gb = gobp.tile([128, 512], BF16, tag="gobf",
                                       name=f"gb{oc}_{st}")
                        nc.vector.tensor_copy(gb, go)
                        nc.sync.dma_start(out=gT[ts(oc, 128), ts(st, 512)],
                                          in_=gb)
                # w2 prefetch: 2 chunks per batch
                for q in range(2):
                    hc = 2 * ocb + q
                    w2c = w2sp.tile([128, D], F32, tag="w2c", name=f"w2c{hc}")
                    nc.sync.dma_start(out=w2c, in_=w2t[ts(hc, 128), :])
                    nc.scalar.activation(w2c, w2c, Copy, bias=C_MAGIC,
                                         scale=scl[:, 5:6])
                    nc.vector.tensor_scalar(out=w2qT[:, hc, :], in0=w2c,
                                            scalar1=C_MAGIC, scalar2=None,
                                            op0=SUB)

            # ---------------- AllGather #2 trigger: global Mg --------------
            garow = misc.tile([128, 4], F32, tag="garow")
            nc.vector.tensor_reduce(out=garow[:, 0:1], in_=gpart, axis=X_AX,
                                    op=MAX)
            for jj in range(1, 4):
                nc.vector.tensor_copy(garow[:, jj:jj + 1], garow[:, 0:1])
            gamax = misc.tile([128, 4], F32, tag="gamax")
            nc.gpsimd.partition_all_reduce(gamax, garow, channels=128,
                                           reduce_op=bass_isa.ReduceOp.max)
            nc.gpsimd.dma_start(out=cc2_in, in_=gamax[0:1, :])
            nc.gpsimd.collective_compute(
                "AllGather", mybir.AluOpType.bypass,
                replica_groups=[list(range(NCORES))],
                ins=[cc2_in.opt()], outs=[cc2_out.opt()])

            g2g = misc.tile([NCORES, 4], F32, tag="g2g")
            nc.gpsimd.dma_start(out=g2g, in_=cc2_out[:, :])
            g2m = misc.tile([NCORES, 4], F32, tag="g2m")
            nc.gpsimd.partition_all_reduce(g2m, g2g, channels=NCORES,
                                           reduce_op=bass_isa.ReduceOp.max)
            g2 = misc.tile([128, 4], F32, tag="g2")
            nc.gpsimd.partition_broadcast(g2, g2m)

            # scl2 cols: 0 sg | 1 inv_sg | 2 d2
            scl2 = misc.tile([128, 4], F32, tag="scl2")
            nc.vector.tensor_scalar(out=scl2[:, 0:1], in0=g2[:, 0:1],
                                    scalar1=1e-8, scalar2=1.0 / QMAX,
                                    op0=MAX, op1=MULT)
            nc.vector.reciprocal(scl2[:, 1:2], scl2[:, 0:1])
            nc.vector.tensor_mul(scl2[:, 2:3], scl2[:, 0:1], scl[:, 4:5])

            # ---------------- fc2: out = gq^T.T @ w2q^T --------------------
            for sc in range(N_SC):
                gsts = []
                for half in range(2):
                    gs = gstp.tile([128, 16, 128], BF16, tag="gst",
                                   name=f"gs{sc}_{half}")
                    nc.sync.dma_start(
                        out=gs,
                        in_=gT[half * 2048:(half + 1) * 2048,
                               ts(sc, 128)].rearrange("(a p) s -> p a s",
                                                      p=128))
                    gsts.append(gs)
                for half in range(2):
                    gf = gsts[half].rearrange("p a b -> p (a b)")
                    qt = qtp.tile([128, 2048], F32, tag="qt",
                                  name=f"gq{sc}_{half}")
                    nc.scalar.activation(qt, gf, Copy, bias=C_MAGIC,
                                         scale=scl2[:, 1:2])
                    nc.vector.tensor_scalar(out=gf, in0=qt, scalar1=C_MAGIC,
                                            scalar2=None, op0=SUB)
                pos = [psump.tile([128, 512], F32, tag="mm",
                                  name=f"po{sc}_{jt}")
                       for jt in range(N_JT)]
                for nn in range(N_NC):
                    for jt in range(N_JT):
                        nc.tensor.matmul(pos[jt],
                                         lhsT=gsts[nn // 16][:, nn % 16, :],
                                         rhs=w2qT[:, nn, ts(jt, 512)],
                                         start=(nn == 0),
                                         stop=(nn == N_NC - 1))
                for jt in range(N_JT):
                    ot = outp.tile([128, 512], F32, tag="ot",
                                   name=f"ot{sc}_{jt}")
                    nc.scalar.activation(ot, pos[jt], Copy, bias=0.0,
                                         scale=scl2[:, 2:3])
                    nc.vector.tensor_add(ot, ot, b2r[:, ts(jt, 512)])
                    nc.sync.dma_start(out=out[ts(sc, 128), ts(jt, 512)],
                                      in_=ot)

    nc.compile()
    return nc


_NC_CACHE = None


def _get_nc():
    global _NC_CACHE
    if _NC_CACHE is None:
        _NC_CACHE = build()
    return _NC_CACHE


def make_in_maps(x, w1, b1, w2, b2):
    xf = np.ascontiguousarray(x.reshape(M, D).T)          # [D, M]
    # w1r[p, ic, h] = w1[h, ic*128+p]
    w1r_h = np.ascontiguousarray(w1.T.reshape(N_IC, 128, H).transpose(1, 0, 2))
    w2t_h = np.ascontiguousarray(w2.T)                    # [H, D]
    b1a_h = np.ascontiguousarray(b1.reshape(N_OC, 128).T)  # [128, 32]
    b2m_h = np.ascontiguousarray(b2.reshape(1, D))
    in_maps = []
    for c in range(NCORES):
        in_maps.append({
            "xts": np.ascontiguousarray(xf[:, c * S:(c + 1) * S]),
            "w1r": w1r_h,
            "w1s": np.ascontiguousarray(
                w1[c * (H // NCORES):(c + 1) * (H // NCORES), :]),
            "w2t": w2t_h,
            "w2s": np.ascontiguousarray(
                w2[c * (D // NCORES):(c + 1) * (D // NCORES), :]),
            "b1a": b1a_h,
            "b2m": b2m_h,
        })
    return in_maps


def kernel(x, w1, b1, w2, b2, _trace=False):
    nc = _get_nc()
    in_maps = make_in_maps(np.asarray(x, dtype=np.float32),
                           np.asarray(w1, dtype=np.float32),
                           np.asarray(b1, dtype=np.float32),
                           np.asarray(w2, dtype=np.float32),
                           np.asarray(b2, dtype=np.float32))
    res = run_bass_kernel_spmd(nc, in_maps, core_ids=list(range(NCORES)),
                               trace=_trace)
    full = np.concatenate([res.results[c]["out"] for c in range(NCORES)],
                          axis=0)
    out = full.reshape(B, T, D)
    if _trace:
        kernel.last_results = res
    return out
